# revision 7
# baseline (speedup 1.0000x reference)
"""Trainium2 Bass kernel for a dense transformer encoder layer (v2).

Reference semantics (B=2, S=2048, D=1024, H=16, DH=64, HID=4096):
    q = einsum('bsd,hde->bhse', x, Wq) + bq          (q == k == v, source bug)
    prob = softmax(q @ q^T / sqrt(DH))
    attn = concat_heads(prob @ q)
    x1 = LN(x + attn);  ff = relu(x1 @ W1 + b1) @ W2 + b2;  out = LN(x1 + ff)

Sharding: core c -> batch c//4, token quarter c%4, rotated so the core's 512
queries sit at rows 0:512 of its 2048-key window (attention is permutation-
equivariant over keys).

All heavy matmuls are fp8e4m3 DoubleRow (256-wide contraction, 2 rows/cycle).
Host reorders feature columns by pi(h,eo,j) = h*64+eo*32+j -> h*64+2j+eo so:
  - qproj-1 emits "qp" [part=32a+j][eo][token], directly usable as both DR
    operands of the q@q^T scores (contraction over dh via eo pairs);
  - qproj-2 (stationary=x^T chunk, moving=packed Wq) emits q token-major,
    giving the wv stationary ("qa", + ones column for the softmax
    denominator) with plain copies -- attention needs no transposes;
  - LN/FFN run in pi-permuted feature order; the host un-permutes the output.
Softmax uses exp(s/8 - 2): the -2 keeps E in fp8 range and cancels in the
normalization.  W2 is weight-split (W2 ~ hi + lo, both fp8) to kill its
weight quantization error at 2x matmul cost.  The v-bias folds into the
residual (host adds bq to x_q); LN's mean-shift invariance lets
y2 = y1*rstd1 + ff skip materializing x1 in f32.

Queries run in two halves so the first half's FFN overlaps the second
half's attention (ACT exp is the critical resource at ~1us per 128x1024
tile; everything else is scheduled around it).
"""

import os

import numpy as np

import concourse.bacc as bacc
import concourse.mybir as mybir
from concourse import tile
from concourse.bass_utils import run_bass_kernel_spmd

dt = mybir.dt
AF = mybir.ActivationFunctionType
ALU = mybir.AluOpType
PM = mybir.MatmulPerfMode

B, S, D = 2, 2048, 1024
H, DH, HID = 16, 64, 4096
SQ = 512            # queries per core
HQ = 256            # queries per half
NCORES = 8
EPS = 1e-5
F32, BF16, FP8 = dt.float32, dt.bfloat16, dt.float8e4

W1_SPLIT = False
W2_SPLIT = True
NW1 = 2 if W1_SPLIT else 1
NW2 = 2 if W2_SPLIT else 1

_BUILD_CACHE = {}
STAGE = 4


def _build(apply_affine: bool):
    if apply_affine in _BUILD_CACHE:
        return _BUILD_CACHE[apply_affine]

    nc = bacc.Bacc("TRN2", target_bir_lowering=False, debug=False,
                   num_devices=NCORES)

    xtp_d = nc.dram_tensor("xtp", [128, 4, 2, S], FP8,
                           kind="ExternalInput").ap()
    xq_d = nc.dram_tensor("xq", [SQ, D], F32, kind="ExternalInput").ap()
    wq1_d = nc.dram_tensor("wq1", [4, 128, 4, 2, 256], FP8,
                           kind="ExternalInput").ap()
    wqm_d = nc.dram_tensor("wqm", [128, 4, 2, D], FP8,
                           kind="ExternalInput").ap()
    bqp_d = nc.dram_tensor("bqp", [128, 8], F32, kind="ExternalInput").ap()
    w1_d = nc.dram_tensor("w1p", [NW1, 8, 128, 4096], FP8,
                          kind="ExternalInput").ap()
    b1_d = nc.dram_tensor("b1r", [128, 32], F32, kind="ExternalInput").ap()
    w2_d = nc.dram_tensor("w2p", [NW2, 8, 128, 4096], FP8,
                          kind="ExternalInput").ap()
    b2_d = nc.dram_tensor("b2bc", [128, D], F32, kind="ExternalInput").ap()
    if apply_affine:
        g1_d = nc.dram_tensor("g1b", [128, D], F32, kind="ExternalInput").ap()
        be1_d = nc.dram_tensor("be1b", [128, D], F32,
                               kind="ExternalInput").ap()
        g2_d = nc.dram_tensor("g2b", [128, D], F32, kind="ExternalInput").ap()
        be2_d = nc.dram_tensor("be2b", [128, D], F32,
                               kind="ExternalInput").ap()
    out_d = nc.dram_tensor("out_q", [SQ, D], F32, kind="ExternalOutput").ap()

    with tile.TileContext(nc) as tc:
        with tc.tile_pool(name="pers", bufs=1) as pp:
            # ---- constants ----
            eps_sb = pp.tile([128, 1], F32, name="eps")
            nc.vector.memset(eps_sb[:], EPS)
            neg2 = pp.tile([128, 1], F32, name="neg2")
            nc.vector.memset(neg2[:], -2.0)
            col_i = pp.tile([128, 128], F32, name="col_i")
            nc.gpsimd.iota(col_i[:], [[1, 128]], channel_multiplier=0,
                           allow_small_or_imprecise_dtypes=True)
            row_i = pp.tile([128, 1], F32, name="row_i")
            nc.gpsimd.iota(row_i[:], [[0, 1]], channel_multiplier=1,
                           allow_small_or_imprecise_dtypes=True)
            idn = pp.tile([128, 128], BF16, name="idn")
            nc.vector.tensor_scalar(idn[:], col_i[:], row_i[:, 0:1], None,
                                    ALU.is_equal)
            idnf = pp.tile([128, 128], F32, name="idnf")
            nc.vector.tensor_scalar(idnf[:], col_i[:], row_i[:, 0:1], None,
                                    ALU.is_equal)

            bqp_sb = pp.tile([128, 8], F32, name="bqp")
            nc.sync.dma_start(bqp_sb[:], bqp_d[:])
            b1_sb = pp.tile([128, 32], F32, name="b1")
            b2_sb = pp.tile([128, D], F32, name="b2")
            if apply_affine:
                g1_sb = pp.tile([128, D], F32, name="g1")
                nc.scalar.dma_start(g1_sb[:], g1_d[:])
                be1_sb = pp.tile([128, D], F32, name="be1")
                nc.scalar.dma_start(be1_sb[:], be1_d[:])
                g2_sb = pp.tile([128, D], F32, name="g2")
                nc.scalar.dma_start(g2_sb[:], g2_d[:])
                be2_sb = pp.tile([128, D], F32, name="be2")
                nc.scalar.dma_start(be2_sb[:], be2_d[:])

            # ---- persistent state ----
            qp = [pp.tile([128, 2, S], FP8, name=f"qp{G}") for G in range(4)]
            qa = [pp.tile([128, 2, 1280], FP8, name=f"qa{c}") for c in range(8)]
            y1 = [pp.tile([128, D], F32, name=f"y1_{b}") for b in range(4)]
            x1fl = [pp.tile([128, D], F32, name=f"x1fl{b}") for b in range(4)]
            z2 = x1fl
            x1tp = [pp.tile([128, 4, 2, HQ], FP8, name=f"x1tp{hf}")
                    for hf in range(2)]
            h1p = [[pp.tile([128, 2, HQ], FP8, name=f"h1p{hf}_{u}")
                    for u in range(16)] for hf in range(2)]
            rstd1 = [pp.tile([128, 1], F32, name=f"rstd1_{b}")
                     for b in range(4)]
            lnscr = pp.tile([128, D], F32, name="lnscr")
            w1_sb = [[pp.tile([128, 4096], FP8, name=f"w1_{sp}_{sl}")
                      for sl in range(8)] for sp in range(NW1)]
            w1v = [[w1_sb[sp][sl].rearrange("p (j c i m) -> p j c i m",
                                            j=4, c=4, i=2)
                    for sl in range(8)] for sp in range(NW1)]
            qa_r = [qa[c].rearrange("p e (h f) -> p e h f", f=80)
                    for c in range(8)]


            # "ones" (+pad) columns of qa -- 32 to match the x32 weight
            # scaling of qa (numerator and denominator stay consistent)
            for cp in range(8):
                nc.gpsimd.memset(qa_r[cp][:, :, :, 64:80], 32.0)

            with (
                tc.tile_pool(name="attn", bufs=1) as ap_,
                tc.tile_pool(name="eppool", bufs=1) as ep,
                tc.tile_pool(name="w2s", bufs=1) as w2s,
                tc.tile_pool(name="scps", bufs=2, space="PSUM") as scps,
                tc.tile_pool(name="tring", bufs=2, space="PSUM") as tring,
                tc.tile_pool(name="fring", bufs=2, space="PSUM") as fring,
            ):
                # late-bound tiles (populated inside the ld scope)
                xtp_sb, wqm_sb, wq1_sb = [], [], []

                # W2 stream: prefetched tag-ring tiles, two m's in flight.
                w2q = {}

                def w2_fetch(half, m):
                    for sp in range(NW2):
                        t = w2s.tile([128, 4096], FP8, tag=f"w2s{sp}", bufs=4,
                                     name=f"w2s{half}_{m}_{sp}")
                        nc.scalar.dma_start(t[:], w2_d[sp, m, :, :])
                        w2q[(half, m, sp)] = t.rearrange(
                            "p (u i m) -> p u i m", u=16, i=2)

                # ---- emission helpers ----
                def emit_qproj1(G):
                    for n in range(4):
                        for eo in range(2):
                            ps = fring.tile([128, 512], F32, tag="fring",
                                            name=f"q1_{G}_{n}_{eo}")
                            for k in range(4):
                                nc.tensor.matmul(
                                    ps[:],
                                    wq1_sb[G][:, k, :,
                                              eo * 128:(eo + 1) * 128],
                                    xtp_sb[k][:, :, n * 512:(n + 1) * 512],
                                    start=(k == 0), stop=(k == 3),
                                    perf_mode=PM.DoubleRow)
                            nc.vector.tensor_scalar_add(
                                qp[G][:, eo, n * 512:(n + 1) * 512], ps[:],
                                bqp_sb[:, 2 * G + eo:2 * G + eo + 1])

                def emit_qproj2(cp):
                    for kk in range(2):
                        cb = 2 * cp + kk
                        for fb in range(2):
                            ps = fring.tile([128, 512], F32, tag="fring",
                                            name=f"q2_{cb}_{fb}")
                            for k in range(4):
                                nc.tensor.matmul(
                                    ps[:],
                                    xtp_sb[k][:, :, cb * 128:(cb + 1) * 128],
                                    wqm_sb[0][:, k, :,
                                              fb * 512:(fb + 1) * 512],
                                    start=(k == 0), stop=(k == 3),
                                    perf_mode=PM.DoubleRow)
                            ps_r = ps.rearrange("p (h f) -> p h f", f=64)
                            nc.vector.tensor_copy(
                                qa_r[cp][:, kk, fb * 8:(fb + 1) * 8, 0:64],
                                ps_r[:, :, :])

                pending_epi = [None]

                def flush_epi():
                    if pending_epi[0] is not None:
                        pending_epi[0]()
                        pending_epi[0] = None

                def emit_attn(half, h, extra=None):
                    G, a = h // 4, h % 4
                    qoff = half * HQ
                    st = qp[G]
                    # uvt[q, (eo,j)+den] accumulated directly in transposed
                    # form: E is the stationary operand, qa the moving one.
                    uvt = tring.tile([128, 2, 256], F32, tag="tring",
                                     name=f"uvt{half}_{h}")
                    Es = []
                    for v in range(4):
                        sc = scps.tile([128, 4, HQ], F32, tag="sc",
                                       name=f"sc{half}_{h}_{v}")
                        for cc in range(4):
                            cb = 4 * v + cc
                            nc.tensor.matmul(
                                sc[:, cc, :],
                                st[32 * a:32 * a + 32, :,
                                   cb * 128:(cb + 1) * 128],
                                st[32 * a:32 * a + 32, :, qoff:qoff + HQ],
                                start=True, stop=True,
                                perf_mode=PM.DoubleRow,
                                tile_position=(32 * a, 0))
                        E = ep.tile([128, 4, HQ], FP8, tag="E", bufs=3,
                                    name=f"E{half}_{h}_{v}")
                        nc.scalar.activation(E[:], sc[:], AF.Exp,
                                             scale=0.125 / 1024.0,
                                             bias=neg2[:, 0:1])
                        Es.append(E)
                        if extra is not None:
                            extra(v)
                        if v == 1:
                            flush_epi()
                        if v > 0:
                            emit_wv(half, h, v - 1, uvt, Es[v - 1])
                    emit_wv(half, h, 3, uvt, Es[3])

                    def epi():
                        rct = ap_.tile([128, 2], F32, tag="rct", bufs=2,
                                       name=f"rct{half}_{h}")
                        nc.vector.reciprocal(rct[:], uvt[:, :, 64])
                        for blk in range(2):
                            yb = y1[2 * half + blk]
                            nc.vector.scalar_tensor_tensor(
                                yb[:, h * 64:(h + 1) * 64],
                                uvt[:, blk, 0:64], rct[:, blk:blk + 1],
                                yb[:, h * 64:(h + 1) * 64],
                                ALU.mult, ALU.add)

                    pending_epi[0] = epi

                def emit_wv(half, h, v, uvt, E):
                    # one accumulation group for both q-blocks: they share a
                    # psum zero region, so only the very first matmul starts
                    # and only the very last stops
                    for w in range(2):
                        cp = 2 * v + w
                        for b in range(2):
                            nc.tensor.matmul(
                                uvt[:, b, 0:80],
                                E[:, 2 * w:2 * w + 2,
                                  b * 128:(b + 1) * 128],
                                qa[cp][:, :, h * 80:h * 80 + 80],
                                start=(v == 0 and w == 0 and b == 0),
                                stop=(v == 3 and w == 1 and b == 1),
                                perf_mode=PM.DoubleRow)

                def emit_ln(half, ln2, tail=False):
                    eng = nc.vector if tail else nc.gpsimd
                    for blk in (2 * half, 2 * half + 1):
                        y = y1[blk]
                        s1 = ap_.tile([128, 1], F32, tag="s1", bufs=2)
                        mean = ap_.tile([128, 1], F32, tag="mean", bufs=2)
                        m2 = ap_.tile([128, 1], F32, tag="m2", bufs=2)
                        if tail and blk % 2 == 1:
                            # ACT is idle after the last exp: offload this
                            # block's stats so the two blocks run in parallel
                            nc.scalar.activation(lnscr[:], y[:], AF.Copy,
                                                 accum_out=s1[:])
                            nc.scalar.activation(lnscr[:], y[:], AF.Square,
                                                 accum_out=m2[:])
                        else:
                            nc.vector.reduce_sum(s1[:], y[:],
                                                 axis=mybir.AxisListType.X)
                            nc.vector.tensor_mul(lnscr[:], y[:], y[:])
                            nc.vector.reduce_sum(m2[:], lnscr[:],
                                                 axis=mybir.AxisListType.X)
                        nc.vector.tensor_scalar_mul(m2[:], m2[:], 1.0 / D)
                        nc.vector.tensor_scalar_mul(mean[:], s1[:], 1.0 / D)
                        msq = ap_.tile([128, 1], F32, tag="msq", bufs=2)
                        nc.vector.tensor_mul(msq[:], mean[:], mean[:])
                        var = ap_.tile([128, 1], F32, tag="var", bufs=2)
                        nc.vector.tensor_sub(var[:], m2[:], msq[:])
                        std = ap_.tile([128, 1], F32, tag="std", bufs=2)
                        nc.scalar.activation(std[:], var[:], AF.Sqrt,
                                             bias=eps_sb[:, 0:1])
                        if ln2:
                            rstd = ap_.tile([128, 1], F32, tag="rstd2",
                                            bufs=2)
                        else:
                            rstd = rstd1[blk]
                        nc.vector.reciprocal(rstd[:], std[:])
                        negm = ap_.tile([128, 1], F32, tag="negm", bufs=2)
                        nc.vector.tensor_scalar_mul(negm[:], mean[:], -1.0)
                        if not ln2:
                            if apply_affine:
                                eng.tensor_scalar(
                                    x1fl[blk][:], y[:], negm[:, 0:1],
                                    rstd[:, 0:1], ALU.add, ALU.mult)
                                eng.tensor_mul(x1fl[blk][:], x1fl[blk][:],
                                               g1_sb[:])
                                eng.tensor_add(x1fl[blk][:], x1fl[blk][:],
                                               be1_sb[:])
                            else:
                                eng.tensor_scalar(
                                    x1fl[blk][:], y[:], negm[:, 0:1],
                                    rstd[:, 0:1], ALU.add, ALU.mult)
                        else:
                            outp = ap_.tile([128, D], F32, tag="outp", bufs=2,
                                            name=f"outp{blk}")
                            eng.tensor_scalar(
                                outp[:], y[:], negm[:, 0:1], rstd[:, 0:1],
                                ALU.add, ALU.mult)
                            if apply_affine:
                                eng.tensor_mul(outp[:], outp[:], g2_sb[:])
                                eng.tensor_add(outp[:], outp[:], be2_sb[:])
                            nc.sync.dma_start(
                                out_d[blk * 128:(blk + 1) * 128, :], outp[:])

                def emit_z2(half):
                    # z2 overwrites x1fl (x1tp already captured x1 in fp8):
                    # z2 = y1*rstd1 + b2, or x1 + b2 in the affine path
                    for blk in (2 * half, 2 * half + 1):
                        if apply_affine:
                            nc.vector.tensor_add(z2[blk][:], x1fl[blk][:],
                                                 b2_sb[:])
                        else:
                            nc.vector.scalar_tensor_tensor(
                                z2[blk][:], y1[blk][:], rstd1[blk][:, 0:1],
                                b2_sb[:], ALU.mult, ALU.add)

                def emit_x1tp_chunk(half, c):
                    for c in (c,):
                        tp = tring.tile([128, 2, HQ], F32, tag="tring",
                                        name=f"x1t{half}_{c}")
                        for i in range(2):
                            for bb in range(2):
                                nc.tensor.transpose(
                                    tp[:, i, bb * 128:(bb + 1) * 128],
                                    x1fl[2 * half + bb][
                                        :, (2 * c + i) * 128:
                                        (2 * c + i + 1) * 128],
                                    idnf[:])
                        nc.vector.tensor_copy(x1tp[half][:, c, :, :], tp[:])

                def emit_w1(half, j):
                    pst = fring.tile([128, 512], F32, tag="fring",
                                     name=f"h1_{half}_{j}")
                    ps = pst[:, 0:HQ]
                    nmm = 4 * NW1
                    n = 0
                    for sp in range(NW1):
                        for c in range(4):
                            nc.tensor.matmul(
                                ps, w1v[sp][j // 4][:, j % 4, c, :, :],
                                x1tp[half][:, c, :, :],
                                start=(n == 0), stop=(n == nmm - 1),
                                perf_mode=PM.DoubleRow)
                            n += 1
                    if half == 0:
                        nc.vector.tensor_scalar(
                            h1p[half][j // 2][:, j % 2, :], ps,
                            b1_sb[:, j:j + 1], 0.0, ALU.add, ALU.max)
                    else:
                        nc.scalar.activation(
                            h1p[half][j // 2][:, j % 2, :], ps, AF.Relu,
                            bias=b1_sb[:, j:j + 1])

                def emit_w2(half, m, blocks=(0, 1)):
                    # transposed output: h1p is the stationary operand, so
                    # ffT arrives as [token, feature] -- no transpose needed.
                    for bb in blocks:
                        blk = 2 * half + bb
                        ff = fring.tile([128, 512], F32, tag="fring",
                                        name=f"ff_{half}_{m}_{bb}")
                        n = 0
                        nmm = 16 * NW2
                        for sp in range(NW2):
                            w2v = w2q[(half, m, sp)]
                            for u in range(16):
                                nc.tensor.matmul(
                                    ff[:, 0:128],
                                    h1p[half][u][:, :,
                                                 bb * 128:(bb + 1) * 128],
                                    w2v[:, u, :, :],
                                    start=(n == 0), stop=(n == nmm - 1),
                                    perf_mode=PM.DoubleRow)
                                n += 1
                        yb = y1[blk]
                        # y2 = z + ff/2048 where z = y1*rstd1 + b2 (or
                        # x1 + b2 in the affine path), precomputed per block
                        nc.vector.scalar_tensor_tensor(
                            yb[:, m * 128:(m + 1) * 128],
                            ff[:, 0:128], 1.0 / 2048.0,
                            z2[blk][:, m * 128:(m + 1) * 128],
                            ALU.mult, ALU.add)

                # ---- schedule ----
                def extra_h0(v):
                    emit_qproj2(2 * v)
                    emit_qproj2(2 * v + 1)

                def emit_qproj1_unit(G, n, eo):
                    ps = fring.tile([128, 512], F32, tag="fring",
                                    name=f"q1_{G}_{n}_{eo}")
                    for k in range(4):
                        nc.tensor.matmul(
                            ps[:],
                            wq1_sb[G][:, k, :, eo * 128:(eo + 1) * 128],
                            xtp_sb[k][:, :, n * 512:(n + 1) * 512],
                            start=(k == 0), stop=(k == 3),
                            perf_mode=PM.DoubleRow)
                    nc.vector.tensor_scalar_add(
                        qp[G][:, eo, n * 512:(n + 1) * 512], ps[:],
                        bqp_sb[:, 2 * G + eo:2 * G + eo + 1])

                def extra_g(G):
                    def f(v):
                        for n in (2 * (v % 2), 2 * (v % 2) + 1):
                            for eo in range(2):
                                if v < 2 or n >= 2:
                                    pass
                        # two units per quad: (n, eo) pairs in quad order
                        units = [(0, 0), (0, 1), (1, 0), (1, 1),
                                 (2, 0), (2, 1), (3, 0), (3, 1)]
                        for n, eo in units[2 * v:2 * v + 2]:
                            emit_qproj1_unit(G, n, eo)
                    return f

                with (
                    tc.tile_pool(name="ld", bufs=1) as ld,
                ):
                    # ---- input loads (latency-ordered) ----
                    t = ld.tile([128, 4, 2, 256], FP8, name="wq1_0")
                    nc.sync.dma_start(t[:], wq1_d[0, :, :, :, :])
                    wq1_sb.append(t)
                    for k in range(4):
                        t = ld.tile([128, 2, S], FP8, name=f"xtp{k}")
                        (nc.sync if k % 2 == 0 else nc.scalar).dma_start(
                            t[:, :, 0:512], xtp_d[:, k, :, 0:512])
                        xtp_sb.append(t)
                    for k in range(4):
                        (nc.sync if k % 2 == 0 else nc.scalar).dma_start(
                            xtp_sb[k][:, :, 512:S], xtp_d[:, k, :, 512:S])
                    t = ld.tile([128, 4, 2, D], FP8, name="wqm")
                    nc.scalar.dma_start(t[:], wqm_d[:])
                    wqm_sb.append(t)
                    for G in range(1, 4):
                        t = ld.tile([128, 4, 2, 256], FP8, name=f"wq1_{G}")
                        nc.sync.dma_start(t[:], wq1_d[G, :, :, :, :])
                        wq1_sb.append(t)
                    # residuals (xq has bq and the pi permutation folded in)
                    for blk in range(4):
                        nc.sync.dma_start(y1[blk][:],
                                          xq_d[blk * 128:(blk + 1) * 128, :])
                    nc.sync.dma_start(b1_sb[:], b1_d[:])
                    nc.sync.dma_start(b2_sb[:], b2_d[:])

                    # half-0 attention heads 0..3, qproj threaded in
                    emit_qproj1(0)
                    if STAGE >= 2:
                        emit_attn(0, 0, extra=extra_h0)
                        for h in (1, 2, 3):
                            emit_attn(0, h, extra=extra_g(h))
                    else:
                        extra_h0(0)
                        extra_h0(1)
                        extra_h0(2)
                        extra_h0(3)
                        for G in (1, 2, 3):
                            emit_qproj1(G)
                        for sp in range(NW1):
                            for sl in range(8):
                                nc.scalar.dma_start(w1_sb[sp][sl][:],
                                                    w1_d[sp, sl, :, :])

                if True:
                    if STAGE >= 2:
                        for h in range(4, H):
                            if 4 <= h < 4 + 8 * NW1:
                                sp, sl = divmod(h - 4, 8)
                                nc.scalar.dma_start(w1_sb[sp][sl][:],
                                                    w1_d[sp, sl, :, :])
                            emit_attn(0, h)
                    SUB = "all"
                    flush_epi()
                    if STAGE >= 3:
                        emit_ln(0, ln2=False)
                    # half-1 attention with half-0 FFN interleaved (the
                    # FFN chunk is emitted mid-head, after quad 2's exp, so
                    # its dep-parked MMs drain while ACT chews quads 2-3)
                    def ffn0_chunk(h, part):
                        if STAGE < 3:
                            return
                        if h == 0:
                            # spread x1tp(0) + z2(0) into head 0's quads so
                            # its dep-parked transposes never block the SEQ
                            if SUB in ("x1tp", "w1", "w2", "all"):
                                emit_x1tp_chunk(0, 2 * part)
                                emit_x1tp_chunk(0, 2 * part + 1)
                                if part == 1:
                                    emit_z2(0)
                            return
                        if h < 9:
                            if part == 0:
                                if h == 7 and SUB in ("w2", "all"):
                                    w2_fetch(0, 0)
                                if h == 8 and SUB in ("w2", "all"):
                                    w2_fetch(0, 1)
                            if SUB in ("w1", "w2", "all"):
                                for j in range(4 * (h - 1) + 2 * part,
                                               4 * (h - 1) + 2 * part + 2):
                                    emit_w1(0, j)
                        else:
                            if SUB in ("w2", "all"):
                                if part == 0 and h < 15:
                                    w2_fetch(0, h - 7)
                                emit_w2(0, h - 9, blocks=(part,))

                    for h in range(H):
                        if STAGE >= 2:
                            done = [0]

                            def mid(v, h=h, done=done):
                                if v in (2, 3) and done[0] == v - 2:
                                    ffn0_chunk(h, v - 2)
                                    done[0] = v - 1
                            emit_attn(1, h, extra=mid)
                            while done[0] < 2:
                                ffn0_chunk(h, done[0])
                                done[0] += 1
                        else:
                            ffn0_chunk(h, 0)
                            ffn0_chunk(h, 1)
                    if STAGE >= 3 and SUB in ("w2", "all"):
                        emit_w2(0, 7)
                    if STAGE >= 4:
                        flush_epi()
                        emit_ln(0, ln2=True)
                        emit_ln(1, ln2=False, tail=True)
                        for c in range(4):
                            emit_x1tp_chunk(1, c)
                        emit_z2(1)
                        for j in range(32):
                            if j % 2 == 0 and j < 8:
                                w2_fetch(1, j // 2)
                            emit_w1(1, j)
                        for m in range(8):
                            if m < 4:
                                w2_fetch(1, m + 4)
                            emit_w2(1, m)
                        emit_ln(1, ln2=True, tail=True)
                    else:
                        # sink: store y1 so the module has outputs
                        for blk in range(4):
                            nc.sync.dma_start(
                                out_d[blk * 128:(blk + 1) * 128, :],
                                y1[blk][:])

    nc.compile()
    _BUILD_CACHE[apply_affine] = nc
    return nc


# ---- host-side packing ----

_PI = np.empty(D, np.int64)
for _h in range(H):
    for _eo in range(2):
        for _j in range(32):
            _PI[_h * 64 + _eo * 32 + _j] = _h * 64 + 2 * _j + _eo


def _f8(a):
    f8 = dt.np(FP8)
    return np.ascontiguousarray(np.asarray(a, np.float32)).astype(f8)


def kernel(x, Wq, bq, ln1_g, ln1_b, W1, b1, W2, b2, ln2_g, ln2_b):
    x = np.asarray(x, np.float32)
    trivial = (np.all(ln1_g == 1) and np.all(ln1_b == 0)
               and np.all(ln2_g == 1) and np.all(ln2_b == 0))
    nc = _build(apply_affine=not trivial)

    Wqf = np.asarray(Wq, np.float32).transpose(1, 0, 2).reshape(D, D)
    bqf = np.asarray(bq, np.float32).reshape(D)
    W1 = np.asarray(W1, np.float32)
    W2 = np.asarray(W2, np.float32)
    b1 = np.asarray(b1, np.float32)
    b2 = np.asarray(b2, np.float32)

    # wq1[G, p, k, i, eo*128 + 32a + j] = Wqf[256k+128i+p, (4G+a)*64 + 2j+eo]
    wq_r = Wqf.reshape(4, 2, 128, H, 32, 2)      # [k, i, p, h, j, eo]
    wq1 = np.transpose(wq_r.reshape(4, 2, 128, 4, 4, 32, 2),
                       (3, 2, 0, 1, 6, 4, 5))    # [G, p, k, i, eo, a, j]
    wq1 = _f8(32.0 * wq1.reshape(4, 128, 4, 2, 256))
    # wqm[p, k, i, f] = Wqf[256k+128i+p, PI[f]]
    wqm = _f8(32.0 * np.transpose(
        Wqf[:, _PI].reshape(4, 2, 128, D), (2, 0, 1, 3)))
    # bqp[32a+j, 2G+eo] = bqf[(4G+a)*64 + 2j+eo]
    bq_r = bqf.reshape(4, 4, 32, 2)              # [G, a, j, eo]
    bqp = np.ascontiguousarray(
        32.0 * np.transpose(bq_r, (1, 2, 0, 3)).reshape(128, 8))

    def wsplit(w, n, scale):
        f8 = dt.np(FP8)
        w = w * scale
        hi = w.astype(f8)
        if n == 1:
            return [hi]
        lo = (w - hi.astype(np.float32)).astype(f8)
        return [hi, lo]

    # w1p[sp, sl, p, (jj c i m)] = W1[PI[256c+128i+p], 128(4sl+jj)+m]
    w1perm = W1[_PI, :]                          # [d(pi), HID]
    w1r = w1perm.reshape(4, 2, 128, 32, 128)     # [c, i, p, j, m]
    w1r = np.transpose(w1r, (2, 3, 0, 1, 4))     # [p, j, c, i, m]
    w1r = np.transpose(w1r.reshape(128, 8, 4, 4, 2, 128),
                       (1, 0, 2, 3, 4, 5))       # [sl, p, jj, c, i, m]
    w1ps = [np.ascontiguousarray(w.reshape(8, 128, 4096))
            for w in wsplit(w1r, NW1, 32.0)]
    # w2p[sp, m, p, (u i mm)] = W2[128(2u+i)+p, PI[128m+mm]]
    w2perm = W2[:, _PI]
    w2r = w2perm.reshape(16, 2, 128, 8, 128)     # [u, i, p, m, mm]
    w2r = np.transpose(w2r, (3, 2, 0, 1, 4))     # [m, p, u, i, mm]
    w2ps = [np.ascontiguousarray(w.reshape(8, 128, 4096))
            for w in wsplit(w2r, NW2, 64.0)]

    base = {
        "wq1": wq1,
        "wqm": wqm,
        "bqp": bqp,
        "w1p": np.stack(w1ps),
        "b1r": np.ascontiguousarray(32.0 * b1.reshape(32, 128).T),
        "w2p": np.stack(w2ps),
        "b2bc": np.ascontiguousarray(
            np.broadcast_to(b2[_PI], (128, D))),
    }
    if not trivial:
        for name, v in (("g1b", np.asarray(ln1_g, np.float32)[_PI]),
                        ("be1b", np.asarray(ln1_b, np.float32)[_PI]),
                        ("g2b", np.asarray(ln2_g, np.float32)[_PI]),
                        ("be2b", np.asarray(ln2_b, np.float32)[_PI])):
            base[name] = np.ascontiguousarray(
                np.broadcast_to(v, (128, D)))

    in_maps = []
    for c in range(NCORES):
        b, t = divmod(c, 4)
        xb = np.concatenate([x[b, t * SQ:], x[b, :t * SQ]], axis=0)
        # xtp[p, k, i, t] = xb[t, 256k+128i+p]
        xtp = _f8(np.transpose(xb.T.reshape(4, 2, 128, S), (2, 0, 1, 3)))
        xq = np.ascontiguousarray(xb[:SQ][:, _PI] + bqf[_PI][None, :])
        in_maps.append({**base, "xtp": xtp, "xq": xq})

    trace = bool(int(os.environ.get("KERNEL_TRACE", "0")))
    kw = {}
    if trace:
        kw = dict(trace=True,
                  tmpdir=os.environ.get("KERNEL_TRACE_DIR") or None)
    res = run_bass_kernel_spmd(nc, in_maps, core_ids=list(range(NCORES)),
                               **kw)
    if trace:
        print(f"HW exec time: {res.exec_time_ns} ns  "
              f"(mean {res.mean_exec_time_ns}, "
              f"max core {res.max_exec_time_core_id})")
    out = np.empty((B, S, D), np.float32)
    inv = np.empty((SQ, D), np.float32)
    for c in range(NCORES):
        b, t = divmod(c, 4)
        inv[:, _PI] = res.results[c]["out_q"]
        out[b, t * SQ:(t + 1) * SQ] = inv
    return out


# revision 8
# speedup vs baseline: 1.0100x; 1.0100x over previous
"""Trainium2 Bass kernel for a dense transformer encoder layer (v2).

Reference semantics (B=2, S=2048, D=1024, H=16, DH=64, HID=4096):
    q = einsum('bsd,hde->bhse', x, Wq) + bq          (q == k == v, source bug)
    prob = softmax(q @ q^T / sqrt(DH))
    attn = concat_heads(prob @ q)
    x1 = LN(x + attn);  ff = relu(x1 @ W1 + b1) @ W2 + b2;  out = LN(x1 + ff)

Sharding: core c -> batch c//4, token quarter c%4, rotated so the core's 512
queries sit at rows 0:512 of its 2048-key window (attention is permutation-
equivariant over keys).

All heavy matmuls are fp8e4m3 DoubleRow (256-wide contraction, 2 rows/cycle).
Host reorders feature columns by pi(h,eo,j) = h*64+eo*32+j -> h*64+2j+eo so:
  - qproj-1 emits "qp" [part=32a+j][eo][token], directly usable as both DR
    operands of the q@q^T scores (contraction over dh via eo pairs);
  - qproj-2 (stationary=x^T chunk, moving=packed Wq) emits q token-major,
    giving the wv stationary ("qa", + ones column for the softmax
    denominator) with plain copies -- attention needs no transposes;
  - LN/FFN run in pi-permuted feature order; the host un-permutes the output.
Softmax uses exp(s/8 - 2): the -2 keeps E in fp8 range and cancels in the
normalization.  W2 is weight-split (W2 ~ hi + lo, both fp8) to kill its
weight quantization error at 2x matmul cost.  The v-bias folds into the
residual (host adds bq to x_q); LN's mean-shift invariance lets
y2 = y1*rstd1 + ff skip materializing x1 in f32.

Queries run in two halves so the first half's FFN overlaps the second
half's attention (ACT exp is the critical resource at ~1us per 128x1024
tile; everything else is scheduled around it).
"""

import os

import numpy as np

import concourse.bacc as bacc
import concourse.mybir as mybir
from concourse import tile
from concourse.bass_utils import run_bass_kernel_spmd

dt = mybir.dt
AF = mybir.ActivationFunctionType
ALU = mybir.AluOpType
PM = mybir.MatmulPerfMode

B, S, D = 2, 2048, 1024
H, DH, HID = 16, 64, 4096
SQ = 512            # queries per core
HQ = 256            # queries per half
NCORES = 8
EPS = 1e-5
F32, BF16, FP8 = dt.float32, dt.bfloat16, dt.float8e4

W1_SPLIT = False
W2_SPLIT = True
NW1 = 2 if W1_SPLIT else 1
NW2 = 2 if W2_SPLIT else 1

_BUILD_CACHE = {}
STAGE = 4


def _build(apply_affine: bool):
    if apply_affine in _BUILD_CACHE:
        return _BUILD_CACHE[apply_affine]

    nc = bacc.Bacc("TRN2", target_bir_lowering=False, debug=False,
                   num_devices=NCORES)

    xtp_d = nc.dram_tensor("xtp", [128, 4, 2, S], FP8,
                           kind="ExternalInput").ap()
    xq_d = nc.dram_tensor("xq", [SQ, D], F32, kind="ExternalInput").ap()
    wq1_d = nc.dram_tensor("wq1", [4, 128, 4, 2, 256], FP8,
                           kind="ExternalInput").ap()
    wqm_d = nc.dram_tensor("wqm", [128, 4, 2, D], FP8,
                           kind="ExternalInput").ap()
    bqp_d = nc.dram_tensor("bqp", [128, 8], F32, kind="ExternalInput").ap()
    w1_d = nc.dram_tensor("w1p", [NW1, 8, 128, 4096], FP8,
                          kind="ExternalInput").ap()
    b1_d = nc.dram_tensor("b1r", [128, 32], F32, kind="ExternalInput").ap()
    w2_d = nc.dram_tensor("w2p", [NW2, 8, 128, 4096], FP8,
                          kind="ExternalInput").ap()
    b2_d = nc.dram_tensor("b2bc", [128, D], F32, kind="ExternalInput").ap()
    if apply_affine:
        g1_d = nc.dram_tensor("g1b", [128, D], F32, kind="ExternalInput").ap()
        be1_d = nc.dram_tensor("be1b", [128, D], F32,
                               kind="ExternalInput").ap()
        g2_d = nc.dram_tensor("g2b", [128, D], F32, kind="ExternalInput").ap()
        be2_d = nc.dram_tensor("be2b", [128, D], F32,
                               kind="ExternalInput").ap()
    out_d = nc.dram_tensor("out_q", [SQ, D], F32, kind="ExternalOutput").ap()

    with tile.TileContext(nc) as tc:
        with tc.tile_pool(name="pers", bufs=1) as pp:
            # ---- constants ----
            eps_sb = pp.tile([128, 1], F32, name="eps")
            nc.vector.memset(eps_sb[:], EPS)
            neg2 = pp.tile([128, 1], F32, name="neg2")
            nc.vector.memset(neg2[:], -2.0)
            col_i = pp.tile([128, 128], F32, name="col_i")
            nc.gpsimd.iota(col_i[:], [[1, 128]], channel_multiplier=0,
                           allow_small_or_imprecise_dtypes=True)
            row_i = pp.tile([128, 1], F32, name="row_i")
            nc.gpsimd.iota(row_i[:], [[0, 1]], channel_multiplier=1,
                           allow_small_or_imprecise_dtypes=True)
            idn = pp.tile([128, 128], BF16, name="idn")
            nc.vector.tensor_scalar(idn[:], col_i[:], row_i[:, 0:1], None,
                                    ALU.is_equal)
            idnf = pp.tile([128, 128], F32, name="idnf")
            nc.vector.tensor_scalar(idnf[:], col_i[:], row_i[:, 0:1], None,
                                    ALU.is_equal)

            bqp_sb = pp.tile([128, 8], F32, name="bqp")
            nc.sync.dma_start(bqp_sb[:], bqp_d[:])
            b1_sb = pp.tile([128, 32], F32, name="b1")
            b2_sb = pp.tile([128, D], F32, name="b2")
            if apply_affine:
                g1_sb = pp.tile([128, D], F32, name="g1")
                nc.scalar.dma_start(g1_sb[:], g1_d[:])
                be1_sb = pp.tile([128, D], F32, name="be1")
                nc.scalar.dma_start(be1_sb[:], be1_d[:])
                g2_sb = pp.tile([128, D], F32, name="g2")
                nc.scalar.dma_start(g2_sb[:], g2_d[:])
                be2_sb = pp.tile([128, D], F32, name="be2")
                nc.scalar.dma_start(be2_sb[:], be2_d[:])

            # ---- persistent state ----
            qp = [pp.tile([128, 2, S], FP8, name=f"qp{G}") for G in range(4)]
            qa = [pp.tile([128, 2, 1280], FP8, name=f"qa{c}") for c in range(8)]
            y1 = [pp.tile([128, D], F32, name=f"y1_{b}") for b in range(4)]
            x1fl = [pp.tile([128, D], F32, name=f"x1fl{b}") for b in range(4)]
            z2 = x1fl
            x1tp = [pp.tile([128, 4, 2, HQ], FP8, name=f"x1tp{hf}")
                    for hf in range(2)]
            h1p = [[pp.tile([128, 2, HQ], FP8, name=f"h1p{hf}_{u}")
                    for u in range(16)] for hf in range(2)]
            rstd1 = [pp.tile([128, 1], F32, name=f"rstd1_{b}")
                     for b in range(4)]
            lnscr = pp.tile([128, D], F32, name="lnscr")
            w1_sb = [[pp.tile([128, 4096], FP8, name=f"w1_{sp}_{sl}")
                      for sl in range(8)] for sp in range(NW1)]
            w1v = [[w1_sb[sp][sl].rearrange("p (j c i m) -> p j c i m",
                                            j=4, c=4, i=2)
                    for sl in range(8)] for sp in range(NW1)]
            qa_r = [qa[c].rearrange("p e (h f) -> p e h f", f=80)
                    for c in range(8)]


            # "ones" (+pad) columns of qa -- 32 to match the x32 weight
            # scaling of qa (numerator and denominator stay consistent)
            for cp in range(8):
                nc.gpsimd.memset(qa_r[cp][:, :, :, 64:80], 32.0)

            with (
                tc.tile_pool(name="attn", bufs=1) as ap_,
                tc.tile_pool(name="eppool", bufs=1) as ep,
                tc.tile_pool(name="w2s", bufs=1) as w2s,
                tc.tile_pool(name="scps", bufs=2, space="PSUM") as scps,
                tc.tile_pool(name="tring", bufs=2, space="PSUM") as tring,
                tc.tile_pool(name="fring", bufs=2, space="PSUM") as fring,
            ):
                # late-bound tiles (populated inside the ld scope)
                xtp_sb, wqm_sb, wq1_sb = [], [], []

                # W2 stream: prefetched tag-ring tiles, two m's in flight.
                w2q = {}

                def w2_fetch(half, m):
                    for sp in range(NW2):
                        t = w2s.tile([128, 4096], FP8, tag=f"w2s{sp}", bufs=4,
                                     name=f"w2s{half}_{m}_{sp}")
                        nc.scalar.dma_start(t[:], w2_d[sp, m, :, :])
                        w2q[(half, m, sp)] = t.rearrange(
                            "p (u i m) -> p u i m", u=16, i=2)

                # ---- emission helpers ----
                def emit_qproj1(G):
                    for n in range(4):
                        for eo in range(2):
                            ps = fring.tile([128, 512], F32, tag="fring",
                                            name=f"q1_{G}_{n}_{eo}")
                            for k in range(4):
                                nc.tensor.matmul(
                                    ps[:],
                                    wq1_sb[G][:, k, :,
                                              eo * 128:(eo + 1) * 128],
                                    xtp_sb[k][:, :, n * 512:(n + 1) * 512],
                                    start=(k == 0), stop=(k == 3),
                                    perf_mode=PM.DoubleRow)
                            nc.vector.tensor_scalar_add(
                                qp[G][:, eo, n * 512:(n + 1) * 512], ps[:],
                                bqp_sb[:, 2 * G + eo:2 * G + eo + 1])

                def emit_qproj2(cp):
                    for kk in range(2):
                        cb = 2 * cp + kk
                        for fb in range(2):
                            ps = fring.tile([128, 512], F32, tag="fring",
                                            name=f"q2_{cb}_{fb}")
                            for k in range(4):
                                nc.tensor.matmul(
                                    ps[:],
                                    xtp_sb[k][:, :, cb * 128:(cb + 1) * 128],
                                    wqm_sb[0][:, k, :,
                                              fb * 512:(fb + 1) * 512],
                                    start=(k == 0), stop=(k == 3),
                                    perf_mode=PM.DoubleRow)
                            ps_r = ps.rearrange("p (h f) -> p h f", f=64)
                            nc.vector.tensor_copy(
                                qa_r[cp][:, kk, fb * 8:(fb + 1) * 8, 0:64],
                                ps_r[:, :, :])

                pending_epi = [None]

                def flush_epi():
                    if pending_epi[0] is not None:
                        pending_epi[0]()
                        pending_epi[0] = None

                def emit_attn(half, h, extra=None):
                    G, a = h // 4, h % 4
                    qoff = half * HQ
                    st = qp[G]
                    # uvt[q, (eo,j)+den] accumulated directly in transposed
                    # form: E is the stationary operand, qa the moving one.
                    uvt = tring.tile([128, 2, 256], F32, tag="tring",
                                     name=f"uvt{half}_{h}")
                    Es = []
                    for v in range(4):
                        sc = scps.tile([128, 4, HQ], F32, tag="sc",
                                       name=f"sc{half}_{h}_{v}")
                        for cc in range(4):
                            cb = 4 * v + cc
                            nc.tensor.matmul(
                                sc[:, cc, :],
                                st[32 * a:32 * a + 32, :,
                                   cb * 128:(cb + 1) * 128],
                                st[32 * a:32 * a + 32, :, qoff:qoff + HQ],
                                start=True, stop=True,
                                perf_mode=PM.DoubleRow,
                                tile_position=(32 * a, 0))
                        E = ep.tile([128, 4, HQ], FP8, tag="E", bufs=3,
                                    name=f"E{half}_{h}_{v}")
                        nc.scalar.activation(E[:], sc[:], AF.Exp,
                                             scale=0.125 / 1024.0,
                                             bias=neg2[:, 0:1])
                        Es.append(E)
                        if extra is not None:
                            extra(v)
                        if v == 1:
                            flush_epi()
                        if v > 0:
                            emit_wv(half, h, v - 1, uvt, Es[v - 1])
                    emit_wv(half, h, 3, uvt, Es[3])

                    def epi():
                        rct = ap_.tile([128, 2], F32, tag="rct", bufs=2,
                                       name=f"rct{half}_{h}")
                        nc.vector.reciprocal(rct[:], uvt[:, :, 64])
                        for blk in range(2):
                            yb = y1[2 * half + blk]
                            nc.vector.scalar_tensor_tensor(
                                yb[:, h * 64:(h + 1) * 64],
                                uvt[:, blk, 0:64], rct[:, blk:blk + 1],
                                yb[:, h * 64:(h + 1) * 64],
                                ALU.mult, ALU.add)

                    pending_epi[0] = epi

                def emit_wv(half, h, v, uvt, E):
                    # one accumulation group for both q-blocks: they share a
                    # psum zero region, so only the very first matmul starts
                    # and only the very last stops
                    for w in range(2):
                        cp = 2 * v + w
                        for b in range(2):
                            nc.tensor.matmul(
                                uvt[:, b, 0:80],
                                E[:, 2 * w:2 * w + 2,
                                  b * 128:(b + 1) * 128],
                                qa[cp][:, :, h * 80:h * 80 + 80],
                                start=(v == 0 and w == 0 and b == 0),
                                stop=(v == 3 and w == 1 and b == 1),
                                perf_mode=PM.DoubleRow)

                def emit_ln(half, ln2, tail=False):
                    eng = nc.vector if tail else nc.gpsimd
                    for blk in (2 * half, 2 * half + 1):
                        y = y1[blk]
                        s1 = ap_.tile([128, 1], F32, tag="s1", bufs=2)
                        mean = ap_.tile([128, 1], F32, tag="mean", bufs=2)
                        m2 = ap_.tile([128, 1], F32, tag="m2", bufs=2)
                        if tail and blk % 2 == 1:
                            # ACT is idle after the last exp: offload this
                            # block's stats so the two blocks run in parallel
                            nc.scalar.activation(lnscr[:], y[:], AF.Copy,
                                                 accum_out=s1[:])
                            nc.scalar.activation(lnscr[:], y[:], AF.Square,
                                                 accum_out=m2[:])
                        else:
                            nc.vector.reduce_sum(s1[:], y[:],
                                                 axis=mybir.AxisListType.X)
                            nc.vector.tensor_mul(lnscr[:], y[:], y[:])
                            nc.vector.reduce_sum(m2[:], lnscr[:],
                                                 axis=mybir.AxisListType.X)
                        nc.vector.tensor_scalar_mul(m2[:], m2[:], 1.0 / D)
                        nc.vector.tensor_scalar_mul(mean[:], s1[:], 1.0 / D)
                        msq = ap_.tile([128, 1], F32, tag="msq", bufs=2)
                        nc.vector.tensor_mul(msq[:], mean[:], mean[:])
                        var = ap_.tile([128, 1], F32, tag="var", bufs=2)
                        nc.vector.tensor_sub(var[:], m2[:], msq[:])
                        std = ap_.tile([128, 1], F32, tag="std", bufs=2)
                        nc.scalar.activation(std[:], var[:], AF.Sqrt,
                                             bias=eps_sb[:, 0:1])
                        if ln2:
                            rstd = ap_.tile([128, 1], F32, tag="rstd2",
                                            bufs=2)
                        else:
                            rstd = rstd1[blk]
                        nc.vector.reciprocal(rstd[:], std[:])
                        negm = ap_.tile([128, 1], F32, tag="negm", bufs=2)
                        nc.vector.tensor_scalar_mul(negm[:], mean[:], -1.0)
                        if not ln2:
                            if apply_affine:
                                eng.tensor_scalar(
                                    x1fl[blk][:], y[:], negm[:, 0:1],
                                    rstd[:, 0:1], ALU.add, ALU.mult)
                                eng.tensor_mul(x1fl[blk][:], x1fl[blk][:],
                                               g1_sb[:])
                                eng.tensor_add(x1fl[blk][:], x1fl[blk][:],
                                               be1_sb[:])
                            else:
                                eng.tensor_scalar(
                                    x1fl[blk][:], y[:], negm[:, 0:1],
                                    rstd[:, 0:1], ALU.add, ALU.mult)
                        else:
                            outp = ap_.tile([128, D], F32, tag="outp", bufs=2,
                                            name=f"outp{blk}")
                            eng.tensor_scalar(
                                outp[:], y[:], negm[:, 0:1], rstd[:, 0:1],
                                ALU.add, ALU.mult)
                            if apply_affine:
                                eng.tensor_mul(outp[:], outp[:], g2_sb[:])
                                eng.tensor_add(outp[:], outp[:], be2_sb[:])
                            nc.sync.dma_start(
                                out_d[blk * 128:(blk + 1) * 128, :], outp[:])

                def emit_z2(half):
                    # z2 overwrites x1fl (x1tp already captured x1 in fp8):
                    # z2 = y1*rstd1 + b2, or x1 + b2 in the affine path
                    for blk in (2 * half, 2 * half + 1):
                        if apply_affine:
                            nc.vector.tensor_add(z2[blk][:], x1fl[blk][:],
                                                 b2_sb[:])
                        else:
                            nc.vector.scalar_tensor_tensor(
                                z2[blk][:], y1[blk][:], rstd1[blk][:, 0:1],
                                b2_sb[:], ALU.mult, ALU.add)

                def emit_x1tp_chunk(half, c):
                    for c in (c,):
                        tp = tring.tile([128, 2, HQ], F32, tag="tring",
                                        name=f"x1t{half}_{c}")
                        for i in range(2):
                            for bb in range(2):
                                nc.tensor.transpose(
                                    tp[:, i, bb * 128:(bb + 1) * 128],
                                    x1fl[2 * half + bb][
                                        :, (2 * c + i) * 128:
                                        (2 * c + i + 1) * 128],
                                    idnf[:])
                        nc.vector.tensor_copy(x1tp[half][:, c, :, :], tp[:])

                def emit_w1(half, j):
                    pst = fring.tile([128, 512], F32, tag="fring",
                                     name=f"h1_{half}_{j}")
                    ps = pst[:, 0:HQ]
                    nmm = 4 * NW1
                    n = 0
                    for sp in range(NW1):
                        for c in range(4):
                            nc.tensor.matmul(
                                ps, w1v[sp][j // 4][:, j % 4, c, :, :],
                                x1tp[half][:, c, :, :],
                                start=(n == 0), stop=(n == nmm - 1),
                                perf_mode=PM.DoubleRow)
                            n += 1
                    if half == 0:
                        nc.vector.tensor_scalar(
                            h1p[half][j // 2][:, j % 2, :], ps,
                            b1_sb[:, j:j + 1], 0.0, ALU.add, ALU.max)
                    else:
                        nc.scalar.activation(
                            h1p[half][j // 2][:, j % 2, :], ps, AF.Relu,
                            bias=b1_sb[:, j:j + 1])

                def emit_w2(half, m, blocks=(0, 1)):
                    # transposed output: h1p is the stationary operand, so
                    # ffT arrives as [token, feature] -- no transpose needed.
                    for bb in blocks:
                        blk = 2 * half + bb
                        ff = fring.tile([128, 512], F32, tag="fring",
                                        name=f"ff_{half}_{m}_{bb}")
                        n = 0
                        nmm = 16 * NW2
                        for sp in range(NW2):
                            w2v = w2q[(half, m, sp)]
                            for u in range(16):
                                nc.tensor.matmul(
                                    ff[:, 0:128],
                                    h1p[half][u][:, :,
                                                 bb * 128:(bb + 1) * 128],
                                    w2v[:, u, :, :],
                                    start=(n == 0), stop=(n == nmm - 1),
                                    perf_mode=PM.DoubleRow)
                                n += 1
                        yb = y1[blk]
                        # y2 = z + ff/2048 where z = y1*rstd1 + b2 (or
                        # x1 + b2 in the affine path), precomputed per block
                        nc.vector.scalar_tensor_tensor(
                            yb[:, m * 128:(m + 1) * 128],
                            ff[:, 0:128], 1.0 / 2048.0,
                            z2[blk][:, m * 128:(m + 1) * 128],
                            ALU.mult, ALU.add)

                # ---- schedule ----
                def extra_h0(v):
                    emit_qproj2(2 * v)
                    emit_qproj2(2 * v + 1)

                def emit_qproj1_unit(G, n, eo):
                    ps = fring.tile([128, 512], F32, tag="fring",
                                    name=f"q1_{G}_{n}_{eo}")
                    for k in range(4):
                        nc.tensor.matmul(
                            ps[:],
                            wq1_sb[G][:, k, :, eo * 128:(eo + 1) * 128],
                            xtp_sb[k][:, :, n * 512:(n + 1) * 512],
                            start=(k == 0), stop=(k == 3),
                            perf_mode=PM.DoubleRow)
                    nc.vector.tensor_scalar_add(
                        qp[G][:, eo, n * 512:(n + 1) * 512], ps[:],
                        bqp_sb[:, 2 * G + eo:2 * G + eo + 1])

                def extra_g(G):
                    def f(v):
                        for n in (2 * (v % 2), 2 * (v % 2) + 1):
                            for eo in range(2):
                                if v < 2 or n >= 2:
                                    pass
                        # two units per quad: (n, eo) pairs in quad order
                        units = [(0, 0), (0, 1), (1, 0), (1, 1),
                                 (2, 0), (2, 1), (3, 0), (3, 1)]
                        for n, eo in units[2 * v:2 * v + 2]:
                            emit_qproj1_unit(G, n, eo)
                    return f

                with (
                    tc.tile_pool(name="ld", bufs=1) as ld,
                ):
                    # PE warm-up: the cost model runs the PE at 0.65/1.2GHz
                    # until it has been continuously busy for 3us. Burn the
                    # initial DMA wait on dependency-free transposes so the
                    # real qproj/scores start at full clock.
                    for i in range(20):
                        wt = fring.tile([128, 512], F32, tag="fring",
                                        name=f"warm{i}")
                        nc.tensor.transpose(wt[:, 0:128], idnf[:], idnf[:])
                    # ---- input loads (latency-ordered) ----
                    t = ld.tile([128, 4, 2, 256], FP8, name="wq1_0")
                    nc.sync.dma_start(t[:], wq1_d[0, :, :, :, :])
                    wq1_sb.append(t)
                    for k in range(4):
                        t = ld.tile([128, 2, S], FP8, name=f"xtp{k}")
                        (nc.sync if k % 2 == 0 else nc.scalar).dma_start(
                            t[:, :, 0:512], xtp_d[:, k, :, 0:512])
                        xtp_sb.append(t)
                    for k in range(4):
                        (nc.sync if k % 2 == 0 else nc.scalar).dma_start(
                            xtp_sb[k][:, :, 512:S], xtp_d[:, k, :, 512:S])
                    t = ld.tile([128, 4, 2, D], FP8, name="wqm")
                    nc.scalar.dma_start(t[:], wqm_d[:])
                    wqm_sb.append(t)
                    for G in range(1, 4):
                        t = ld.tile([128, 4, 2, 256], FP8, name=f"wq1_{G}")
                        nc.sync.dma_start(t[:], wq1_d[G, :, :, :, :])
                        wq1_sb.append(t)
                    # residuals (xq has bq and the pi permutation folded in)
                    for blk in range(4):
                        nc.sync.dma_start(y1[blk][:],
                                          xq_d[blk * 128:(blk + 1) * 128, :])
                    nc.sync.dma_start(b1_sb[:], b1_d[:])
                    nc.sync.dma_start(b2_sb[:], b2_d[:])

                    # half-0 attention heads 0..3, qproj threaded in
                    emit_qproj1(0)
                    if STAGE >= 2:
                        emit_attn(0, 0, extra=extra_h0)
                        for h in (1, 2, 3):
                            emit_attn(0, h, extra=extra_g(h))
                    else:
                        extra_h0(0)
                        extra_h0(1)
                        extra_h0(2)
                        extra_h0(3)
                        for G in (1, 2, 3):
                            emit_qproj1(G)
                        for sp in range(NW1):
                            for sl in range(8):
                                nc.scalar.dma_start(w1_sb[sp][sl][:],
                                                    w1_d[sp, sl, :, :])

                if True:
                    if STAGE >= 2:
                        for h in range(4, H):
                            if 4 <= h < 4 + 8 * NW1:
                                sp, sl = divmod(h - 4, 8)
                                nc.scalar.dma_start(w1_sb[sp][sl][:],
                                                    w1_d[sp, sl, :, :])
                            emit_attn(0, h)
                    SUB = "all"
                    flush_epi()
                    if STAGE >= 3:
                        emit_ln(0, ln2=False)
                    # half-1 attention with half-0 FFN interleaved (the
                    # FFN chunk is emitted mid-head, after quad 2's exp, so
                    # its dep-parked MMs drain while ACT chews quads 2-3)
                    def ffn0_chunk(h, part):
                        if STAGE < 3:
                            return
                        if h == 0:
                            # spread x1tp(0) + z2(0) into head 0's quads so
                            # its dep-parked transposes never block the SEQ
                            if SUB in ("x1tp", "w1", "w2", "all"):
                                emit_x1tp_chunk(0, 2 * part)
                                emit_x1tp_chunk(0, 2 * part + 1)
                                if part == 1:
                                    emit_z2(0)
                            return
                        if h < 9:
                            if part == 0:
                                if h == 7 and SUB in ("w2", "all"):
                                    w2_fetch(0, 0)
                                if h == 8 and SUB in ("w2", "all"):
                                    w2_fetch(0, 1)
                            if SUB in ("w1", "w2", "all"):
                                for j in range(4 * (h - 1) + 2 * part,
                                               4 * (h - 1) + 2 * part + 2):
                                    emit_w1(0, j)
                        else:
                            if SUB in ("w2", "all"):
                                if part == 0 and h < 15:
                                    w2_fetch(0, h - 7)
                                emit_w2(0, h - 9, blocks=(part,))

                    for h in range(H):
                        if STAGE >= 2:
                            done = [0]

                            def mid(v, h=h, done=done):
                                if v in (2, 3) and done[0] == v - 2:
                                    ffn0_chunk(h, v - 2)
                                    done[0] = v - 1
                            emit_attn(1, h, extra=mid)
                            while done[0] < 2:
                                ffn0_chunk(h, done[0])
                                done[0] += 1
                        else:
                            ffn0_chunk(h, 0)
                            ffn0_chunk(h, 1)
                    if STAGE >= 3 and SUB in ("w2", "all"):
                        emit_w2(0, 7)
                    if STAGE >= 4:
                        flush_epi()
                        emit_ln(0, ln2=True)
                        emit_ln(1, ln2=False, tail=True)
                        for c in range(4):
                            emit_x1tp_chunk(1, c)
                        emit_z2(1)
                        for j in range(32):
                            if j % 2 == 0 and j < 8:
                                w2_fetch(1, j // 2)
                            emit_w1(1, j)
                        for m in range(8):
                            if m < 4:
                                w2_fetch(1, m + 4)
                            emit_w2(1, m)
                        emit_ln(1, ln2=True, tail=True)
                    else:
                        # sink: store y1 so the module has outputs
                        for blk in range(4):
                            nc.sync.dma_start(
                                out_d[blk * 128:(blk + 1) * 128, :],
                                y1[blk][:])

    nc.compile()
    _BUILD_CACHE[apply_affine] = nc
    return nc


# ---- host-side packing ----

_PI = np.empty(D, np.int64)
for _h in range(H):
    for _eo in range(2):
        for _j in range(32):
            _PI[_h * 64 + _eo * 32 + _j] = _h * 64 + 2 * _j + _eo


def _f8(a):
    f8 = dt.np(FP8)
    return np.ascontiguousarray(np.asarray(a, np.float32)).astype(f8)


def kernel(x, Wq, bq, ln1_g, ln1_b, W1, b1, W2, b2, ln2_g, ln2_b):
    x = np.asarray(x, np.float32)
    trivial = (np.all(ln1_g == 1) and np.all(ln1_b == 0)
               and np.all(ln2_g == 1) and np.all(ln2_b == 0))
    nc = _build(apply_affine=not trivial)

    Wqf = np.asarray(Wq, np.float32).transpose(1, 0, 2).reshape(D, D)
    bqf = np.asarray(bq, np.float32).reshape(D)
    W1 = np.asarray(W1, np.float32)
    W2 = np.asarray(W2, np.float32)
    b1 = np.asarray(b1, np.float32)
    b2 = np.asarray(b2, np.float32)

    # wq1[G, p, k, i, eo*128 + 32a + j] = Wqf[256k+128i+p, (4G+a)*64 + 2j+eo]
    wq_r = Wqf.reshape(4, 2, 128, H, 32, 2)      # [k, i, p, h, j, eo]
    wq1 = np.transpose(wq_r.reshape(4, 2, 128, 4, 4, 32, 2),
                       (3, 2, 0, 1, 6, 4, 5))    # [G, p, k, i, eo, a, j]
    wq1 = _f8(32.0 * wq1.reshape(4, 128, 4, 2, 256))
    # wqm[p, k, i, f] = Wqf[256k+128i+p, PI[f]]
    wqm = _f8(32.0 * np.transpose(
        Wqf[:, _PI].reshape(4, 2, 128, D), (2, 0, 1, 3)))
    # bqp[32a+j, 2G+eo] = bqf[(4G+a)*64 + 2j+eo]
    bq_r = bqf.reshape(4, 4, 32, 2)              # [G, a, j, eo]
    bqp = np.ascontiguousarray(
        32.0 * np.transpose(bq_r, (1, 2, 0, 3)).reshape(128, 8))

    def wsplit(w, n, scale):
        f8 = dt.np(FP8)
        w = w * scale
        hi = w.astype(f8)
        if n == 1:
            return [hi]
        lo = (w - hi.astype(np.float32)).astype(f8)
        return [hi, lo]

    # w1p[sp, sl, p, (jj c i m)] = W1[PI[256c+128i+p], 128(4sl+jj)+m]
    w1perm = W1[_PI, :]                          # [d(pi), HID]
    w1r = w1perm.reshape(4, 2, 128, 32, 128)     # [c, i, p, j, m]
    w1r = np.transpose(w1r, (2, 3, 0, 1, 4))     # [p, j, c, i, m]
    w1r = np.transpose(w1r.reshape(128, 8, 4, 4, 2, 128),
                       (1, 0, 2, 3, 4, 5))       # [sl, p, jj, c, i, m]
    w1ps = [np.ascontiguousarray(w.reshape(8, 128, 4096))
            for w in wsplit(w1r, NW1, 32.0)]
    # w2p[sp, m, p, (u i mm)] = W2[128(2u+i)+p, PI[128m+mm]]
    w2perm = W2[:, _PI]
    w2r = w2perm.reshape(16, 2, 128, 8, 128)     # [u, i, p, m, mm]
    w2r = np.transpose(w2r, (3, 2, 0, 1, 4))     # [m, p, u, i, mm]
    w2ps = [np.ascontiguousarray(w.reshape(8, 128, 4096))
            for w in wsplit(w2r, NW2, 64.0)]

    base = {
        "wq1": wq1,
        "wqm": wqm,
        "bqp": bqp,
        "w1p": np.stack(w1ps),
        "b1r": np.ascontiguousarray(32.0 * b1.reshape(32, 128).T),
        "w2p": np.stack(w2ps),
        "b2bc": np.ascontiguousarray(
            np.broadcast_to(b2[_PI], (128, D))),
    }
    if not trivial:
        for name, v in (("g1b", np.asarray(ln1_g, np.float32)[_PI]),
                        ("be1b", np.asarray(ln1_b, np.float32)[_PI]),
                        ("g2b", np.asarray(ln2_g, np.float32)[_PI]),
                        ("be2b", np.asarray(ln2_b, np.float32)[_PI])):
            base[name] = np.ascontiguousarray(
                np.broadcast_to(v, (128, D)))

    in_maps = []
    for c in range(NCORES):
        b, t = divmod(c, 4)
        xb = np.concatenate([x[b, t * SQ:], x[b, :t * SQ]], axis=0)
        # xtp[p, k, i, t] = xb[t, 256k+128i+p]
        xtp = _f8(np.transpose(xb.T.reshape(4, 2, 128, S), (2, 0, 1, 3)))
        xq = np.ascontiguousarray(xb[:SQ][:, _PI] + bqf[_PI][None, :])
        in_maps.append({**base, "xtp": xtp, "xq": xq})

    trace = bool(int(os.environ.get("KERNEL_TRACE", "0")))
    kw = {}
    if trace:
        kw = dict(trace=True,
                  tmpdir=os.environ.get("KERNEL_TRACE_DIR") or None)
    res = run_bass_kernel_spmd(nc, in_maps, core_ids=list(range(NCORES)),
                               **kw)
    if trace:
        print(f"HW exec time: {res.exec_time_ns} ns  "
              f"(mean {res.mean_exec_time_ns}, "
              f"max core {res.max_exec_time_core_id})")
    out = np.empty((B, S, D), np.float32)
    inv = np.empty((SQ, D), np.float32)
    for c in range(NCORES):
        b, t = divmod(c, 4)
        inv[:, _PI] = res.results[c]["out_q"]
        out[b, t * SQ:(t + 1) * SQ] = inv
    return out


# revision 9
# speedup vs baseline: 1.0153x; 1.0052x over previous
"""Trainium2 Bass kernel for a dense transformer encoder layer (v2).

Reference semantics (B=2, S=2048, D=1024, H=16, DH=64, HID=4096):
    q = einsum('bsd,hde->bhse', x, Wq) + bq          (q == k == v, source bug)
    prob = softmax(q @ q^T / sqrt(DH))
    attn = concat_heads(prob @ q)
    x1 = LN(x + attn);  ff = relu(x1 @ W1 + b1) @ W2 + b2;  out = LN(x1 + ff)

Sharding: core c -> batch c//4, token quarter c%4, rotated so the core's 512
queries sit at rows 0:512 of its 2048-key window (attention is permutation-
equivariant over keys).

All heavy matmuls are fp8e4m3 DoubleRow (256-wide contraction, 2 rows/cycle).
Host reorders feature columns by pi(h,eo,j) = h*64+eo*32+j -> h*64+2j+eo so:
  - qproj-1 emits "qp" [part=32a+j][eo][token], directly usable as both DR
    operands of the q@q^T scores (contraction over dh via eo pairs);
  - qproj-2 (stationary=x^T chunk, moving=packed Wq) emits q token-major,
    giving the wv stationary ("qa", + ones column for the softmax
    denominator) with plain copies -- attention needs no transposes;
  - LN/FFN run in pi-permuted feature order; the host un-permutes the output.
Softmax uses exp(s/8 - 2): the -2 keeps E in fp8 range and cancels in the
normalization.  W2 is weight-split (W2 ~ hi + lo, both fp8) to kill its
weight quantization error at 2x matmul cost.  The v-bias folds into the
residual (host adds bq to x_q); LN's mean-shift invariance lets
y2 = y1*rstd1 + ff skip materializing x1 in f32.

Queries run in two halves so the first half's FFN overlaps the second
half's attention (ACT exp is the critical resource at ~1us per 128x1024
tile; everything else is scheduled around it).
"""

import os

import numpy as np

import concourse.bacc as bacc
import concourse.mybir as mybir
from concourse import tile
from concourse.bass_utils import run_bass_kernel_spmd

dt = mybir.dt
AF = mybir.ActivationFunctionType
ALU = mybir.AluOpType
PM = mybir.MatmulPerfMode

B, S, D = 2, 2048, 1024
H, DH, HID = 16, 64, 4096
SQ = 512            # queries per core
HQ = 256            # queries per half
NCORES = 8
EPS = 1e-5
F32, BF16, FP8 = dt.float32, dt.bfloat16, dt.float8e4

W1_SPLIT = False
W2_SPLIT = True
NW1 = 2 if W1_SPLIT else 1
NW2 = 2 if W2_SPLIT else 1

_BUILD_CACHE = {}
STAGE = 4


def _build(apply_affine: bool):
    if apply_affine in _BUILD_CACHE:
        return _BUILD_CACHE[apply_affine]

    nc = bacc.Bacc("TRN2", target_bir_lowering=False, debug=False,
                   num_devices=NCORES)

    xtp_d = nc.dram_tensor("xtp", [128, 4, 2, S], FP8,
                           kind="ExternalInput").ap()
    xq_d = nc.dram_tensor("xq", [SQ, D], F32, kind="ExternalInput").ap()
    wq1_d = nc.dram_tensor("wq1", [4, 128, 4, 2, 256], FP8,
                           kind="ExternalInput").ap()
    wqm_d = nc.dram_tensor("wqm", [128, 4, 2, D], FP8,
                           kind="ExternalInput").ap()
    bqp_d = nc.dram_tensor("bqp", [128, 8], F32, kind="ExternalInput").ap()
    w1_d = nc.dram_tensor("w1p", [NW1, 8, 128, 4096], FP8,
                          kind="ExternalInput").ap()
    b1_d = nc.dram_tensor("b1r", [128, 32], F32, kind="ExternalInput").ap()
    w2_d = nc.dram_tensor("w2p", [NW2, 8, 128, 4096], FP8,
                          kind="ExternalInput").ap()
    b2_d = nc.dram_tensor("b2bc", [128, D], F32, kind="ExternalInput").ap()
    if apply_affine:
        g1_d = nc.dram_tensor("g1b", [128, D], F32, kind="ExternalInput").ap()
        be1_d = nc.dram_tensor("be1b", [128, D], F32,
                               kind="ExternalInput").ap()
        g2_d = nc.dram_tensor("g2b", [128, D], F32, kind="ExternalInput").ap()
        be2_d = nc.dram_tensor("be2b", [128, D], F32,
                               kind="ExternalInput").ap()
    out_d = nc.dram_tensor("out_q", [SQ, D], F32, kind="ExternalOutput").ap()

    with tile.TileContext(nc) as tc:
        with tc.tile_pool(name="pers", bufs=1) as pp:
            # ---- constants ----
            eps_sb = pp.tile([128, 1], F32, name="eps")
            nc.vector.memset(eps_sb[:], EPS)
            neg2 = pp.tile([128, 1], F32, name="neg2")
            nc.vector.memset(neg2[:], -2.0)
            col_i = pp.tile([128, 128], F32, name="col_i")
            nc.gpsimd.iota(col_i[:], [[1, 128]], channel_multiplier=0,
                           allow_small_or_imprecise_dtypes=True)
            row_i = pp.tile([128, 1], F32, name="row_i")
            nc.gpsimd.iota(row_i[:], [[0, 1]], channel_multiplier=1,
                           allow_small_or_imprecise_dtypes=True)
            idn = pp.tile([128, 128], BF16, name="idn")
            nc.vector.tensor_scalar(idn[:], col_i[:], row_i[:, 0:1], None,
                                    ALU.is_equal)
            idnf = pp.tile([128, 128], F32, name="idnf")
            nc.vector.tensor_scalar(idnf[:], col_i[:], row_i[:, 0:1], None,
                                    ALU.is_equal)

            bqp_sb = pp.tile([128, 8], F32, name="bqp")
            nc.sync.dma_start(bqp_sb[:], bqp_d[:])
            b1_sb = pp.tile([128, 32], F32, name="b1")
            b2_sb = pp.tile([128, D], F32, name="b2")
            if apply_affine:
                g1_sb = pp.tile([128, D], F32, name="g1")
                nc.scalar.dma_start(g1_sb[:], g1_d[:])
                be1_sb = pp.tile([128, D], F32, name="be1")
                nc.scalar.dma_start(be1_sb[:], be1_d[:])
                g2_sb = pp.tile([128, D], F32, name="g2")
                nc.scalar.dma_start(g2_sb[:], g2_d[:])
                be2_sb = pp.tile([128, D], F32, name="be2")
                nc.scalar.dma_start(be2_sb[:], be2_d[:])

            # ---- persistent state ----
            qp = [pp.tile([128, 2, S], FP8, name=f"qp{G}") for G in range(4)]
            qa = [pp.tile([128, 2, 1280], FP8, name=f"qa{c}") for c in range(8)]
            y1 = [pp.tile([128, D], F32, name=f"y1_{b}") for b in range(4)]
            x1fl = [pp.tile([128, D], F32, name=f"x1fl{b}") for b in range(4)]
            z2 = x1fl
            x1tp = [pp.tile([128, 4, 2, HQ], FP8, name=f"x1tp{hf}")
                    for hf in range(2)]
            h1p = [[pp.tile([128, 2, HQ], FP8, name=f"h1p{hf}_{u}")
                    for u in range(16)] for hf in range(2)]
            rstd1 = [pp.tile([128, 1], F32, name=f"rstd1_{b}")
                     for b in range(4)]
            lnscr = pp.tile([128, D], F32, name="lnscr")
            w1_sb = [[pp.tile([128, 4096], FP8, name=f"w1_{sp}_{sl}")
                      for sl in range(8)] for sp in range(NW1)]
            w1v = [[w1_sb[sp][sl].rearrange("p (j c i m) -> p j c i m",
                                            j=4, c=4, i=2)
                    for sl in range(8)] for sp in range(NW1)]
            qa_r = [qa[c].rearrange("p e (h f) -> p e h f", f=80)
                    for c in range(8)]


            # "ones" (+pad) columns of qa -- 32 to match the x32 weight
            # scaling of qa (numerator and denominator stay consistent)
            for cp in range(8):
                nc.gpsimd.memset(qa_r[cp][:, :, :, 64:80], 32.0)

            with (
                tc.tile_pool(name="attn", bufs=1) as ap_,
                tc.tile_pool(name="eppool", bufs=1) as ep,
                tc.tile_pool(name="w2s", bufs=1) as w2s,
                tc.tile_pool(name="scps", bufs=2, space="PSUM") as scps,
                tc.tile_pool(name="tring", bufs=2, space="PSUM") as tring,
                tc.tile_pool(name="fring", bufs=2, space="PSUM") as fring,
            ):
                # late-bound tiles (populated inside the ld scope)
                xtp_sb, wqm_sb, wq1_sb = [], [], []

                # W2 stream: prefetched tag-ring tiles, two m's in flight.
                w2q = {}

                def w2_fetch(half, m):
                    for sp in range(NW2):
                        t = w2s.tile([128, 4096], FP8, tag=f"w2s{sp}", bufs=4,
                                     name=f"w2s{half}_{m}_{sp}")
                        nc.scalar.dma_start(t[:], w2_d[sp, m, :, :])
                        w2q[(half, m, sp)] = t.rearrange(
                            "p (u i m) -> p u i m", u=16, i=2)

                # ---- emission helpers ----
                def emit_qproj1(G):
                    for n in range(4):
                        for eo in range(2):
                            ps = fring.tile([128, 512], F32, tag="fring",
                                            name=f"q1_{G}_{n}_{eo}")
                            for k in range(4):
                                nc.tensor.matmul(
                                    ps[:],
                                    wq1_sb[G][:, k, :,
                                              eo * 128:(eo + 1) * 128],
                                    xtp_sb[k][:, :, n * 512:(n + 1) * 512],
                                    start=(k == 0), stop=(k == 3),
                                    perf_mode=PM.DoubleRow)
                            nc.vector.tensor_scalar_add(
                                qp[G][:, eo, n * 512:(n + 1) * 512], ps[:],
                                bqp_sb[:, 2 * G + eo:2 * G + eo + 1])

                def emit_qproj2(cp):
                    for kk in range(2):
                        cb = 2 * cp + kk
                        for fb in range(2):
                            ps = fring.tile([128, 512], F32, tag="fring",
                                            name=f"q2_{cb}_{fb}")
                            for k in range(4):
                                nc.tensor.matmul(
                                    ps[:],
                                    xtp_sb[k][:, :, cb * 128:(cb + 1) * 128],
                                    wqm_sb[0][:, k, :,
                                              fb * 512:(fb + 1) * 512],
                                    start=(k == 0), stop=(k == 3),
                                    perf_mode=PM.DoubleRow)
                            ps_r = ps.rearrange("p (h f) -> p h f", f=64)
                            nc.vector.tensor_copy(
                                qa_r[cp][:, kk, fb * 8:(fb + 1) * 8, 0:64],
                                ps_r[:, :, :])

                pending_epi = [None]

                def flush_epi():
                    if pending_epi[0] is not None:
                        pending_epi[0]()
                        pending_epi[0] = None

                def emit_attn(half, h, extra=None):
                    G, a = h // 4, h % 4
                    qoff = half * HQ
                    st = qp[G]
                    # uvt[q, (eo,j)+den] accumulated directly in transposed
                    # form: E is the stationary operand, qa the moving one.
                    uvt = tring.tile([128, 2, 256], F32, tag="tring",
                                     name=f"uvt{half}_{h}")
                    Es = []
                    for v in range(4):
                        sc = scps.tile([128, 4, HQ], F32, tag="sc",
                                       name=f"sc{half}_{h}_{v}")
                        for cc in range(4):
                            cb = 4 * v + cc
                            nc.tensor.matmul(
                                sc[:, cc, :],
                                st[32 * a:32 * a + 32, :,
                                   cb * 128:(cb + 1) * 128],
                                st[32 * a:32 * a + 32, :, qoff:qoff + HQ],
                                start=True, stop=True,
                                perf_mode=PM.DoubleRow,
                                tile_position=(32 * a, 0))
                        E = ep.tile([128, 4, HQ], FP8, tag="E", bufs=5,
                                    name=f"E{half}_{h}_{v}")
                        nc.scalar.activation(E[:], sc[:], AF.Exp,
                                             scale=0.125 / 1024.0,
                                             bias=neg2[:, 0:1])
                        Es.append(E)
                        if extra is not None:
                            extra(v)
                        if v == 1:
                            flush_epi()
                        if v > 0:
                            emit_wv(half, h, v - 1, uvt, Es[v - 1])
                    emit_wv(half, h, 3, uvt, Es[3])

                    def epi():
                        rct = ap_.tile([128, 2], F32, tag="rct", bufs=2,
                                       name=f"rct{half}_{h}")
                        nc.vector.reciprocal(rct[:], uvt[:, :, 64])
                        for blk in range(2):
                            yb = y1[2 * half + blk]
                            nc.vector.scalar_tensor_tensor(
                                yb[:, h * 64:(h + 1) * 64],
                                uvt[:, blk, 0:64], rct[:, blk:blk + 1],
                                yb[:, h * 64:(h + 1) * 64],
                                ALU.mult, ALU.add)

                    pending_epi[0] = epi

                def emit_wv(half, h, v, uvt, E):
                    # one accumulation group for both q-blocks: they share a
                    # psum zero region, so only the very first matmul starts
                    # and only the very last stops
                    for w in range(2):
                        cp = 2 * v + w
                        for b in range(2):
                            nc.tensor.matmul(
                                uvt[:, b, 0:80],
                                E[:, 2 * w:2 * w + 2,
                                  b * 128:(b + 1) * 128],
                                qa[cp][:, :, h * 80:h * 80 + 80],
                                start=(v == 0 and w == 0 and b == 0),
                                stop=(v == 3 and w == 1 and b == 1),
                                perf_mode=PM.DoubleRow)

                def emit_ln(half, ln2, tail=False):
                    eng = nc.vector if tail else nc.gpsimd
                    for blk in (2 * half, 2 * half + 1):
                        y = y1[blk]
                        s1 = ap_.tile([128, 1], F32, tag="s1", bufs=2)
                        mean = ap_.tile([128, 1], F32, tag="mean", bufs=2)
                        m2 = ap_.tile([128, 1], F32, tag="m2", bufs=2)
                        if tail and blk % 2 == 1:
                            # ACT is idle after the last exp: offload this
                            # block's stats so the two blocks run in parallel
                            nc.scalar.activation(lnscr[:], y[:], AF.Copy,
                                                 accum_out=s1[:])
                            nc.scalar.activation(lnscr[:], y[:], AF.Square,
                                                 accum_out=m2[:])
                        else:
                            nc.vector.reduce_sum(s1[:], y[:],
                                                 axis=mybir.AxisListType.X)
                            nc.vector.tensor_mul(lnscr[:], y[:], y[:])
                            nc.vector.reduce_sum(m2[:], lnscr[:],
                                                 axis=mybir.AxisListType.X)
                        nc.vector.tensor_scalar_mul(m2[:], m2[:], 1.0 / D)
                        nc.vector.tensor_scalar_mul(mean[:], s1[:], 1.0 / D)
                        msq = ap_.tile([128, 1], F32, tag="msq", bufs=2)
                        nc.vector.tensor_mul(msq[:], mean[:], mean[:])
                        var = ap_.tile([128, 1], F32, tag="var", bufs=2)
                        nc.vector.tensor_sub(var[:], m2[:], msq[:])
                        std = ap_.tile([128, 1], F32, tag="std", bufs=2)
                        nc.scalar.activation(std[:], var[:], AF.Sqrt,
                                             bias=eps_sb[:, 0:1])
                        if ln2:
                            rstd = ap_.tile([128, 1], F32, tag="rstd2",
                                            bufs=2)
                        else:
                            rstd = rstd1[blk]
                        nc.vector.reciprocal(rstd[:], std[:])
                        negm = ap_.tile([128, 1], F32, tag="negm", bufs=2)
                        nc.vector.tensor_scalar_mul(negm[:], mean[:], -1.0)
                        if not ln2:
                            if apply_affine:
                                eng.tensor_scalar(
                                    x1fl[blk][:], y[:], negm[:, 0:1],
                                    rstd[:, 0:1], ALU.add, ALU.mult)
                                eng.tensor_mul(x1fl[blk][:], x1fl[blk][:],
                                               g1_sb[:])
                                eng.tensor_add(x1fl[blk][:], x1fl[blk][:],
                                               be1_sb[:])
                            else:
                                eng.tensor_scalar(
                                    x1fl[blk][:], y[:], negm[:, 0:1],
                                    rstd[:, 0:1], ALU.add, ALU.mult)
                        else:
                            outp = ap_.tile([128, D], F32, tag="outp", bufs=2,
                                            name=f"outp{blk}")
                            neng = (nc.gpsimd if (tail and blk % 2 == 1)
                                    else eng)
                            neng.tensor_scalar(
                                outp[:], y[:], negm[:, 0:1], rstd[:, 0:1],
                                ALU.add, ALU.mult)
                            if apply_affine:
                                eng.tensor_mul(outp[:], outp[:], g2_sb[:])
                                eng.tensor_add(outp[:], outp[:], be2_sb[:])
                            nc.sync.dma_start(
                                out_d[blk * 128:(blk + 1) * 128, :], outp[:])

                def emit_z2(half):
                    # z2 overwrites x1fl (x1tp already captured x1 in fp8):
                    # z2 = y1*rstd1 + b2, or x1 + b2 in the affine path
                    for blk in (2 * half, 2 * half + 1):
                        if apply_affine:
                            nc.vector.tensor_add(z2[blk][:], x1fl[blk][:],
                                                 b2_sb[:])
                        else:
                            nc.vector.scalar_tensor_tensor(
                                z2[blk][:], y1[blk][:], rstd1[blk][:, 0:1],
                                b2_sb[:], ALU.mult, ALU.add)

                def emit_x1tp_chunk(half, c):
                    for c in (c,):
                        tp = tring.tile([128, 2, HQ], F32, tag="tring",
                                        name=f"x1t{half}_{c}")
                        for i in range(2):
                            for bb in range(2):
                                nc.tensor.transpose(
                                    tp[:, i, bb * 128:(bb + 1) * 128],
                                    x1fl[2 * half + bb][
                                        :, (2 * c + i) * 128:
                                        (2 * c + i + 1) * 128],
                                    idnf[:])
                        nc.vector.tensor_copy(x1tp[half][:, c, :, :], tp[:])

                def emit_w1(half, j):
                    pst = fring.tile([128, 512], F32, tag="fring",
                                     name=f"h1_{half}_{j}")
                    ps = pst[:, 0:HQ]
                    nmm = 4 * NW1
                    n = 0
                    for sp in range(NW1):
                        for c in range(4):
                            nc.tensor.matmul(
                                ps, w1v[sp][j // 4][:, j % 4, c, :, :],
                                x1tp[half][:, c, :, :],
                                start=(n == 0), stop=(n == nmm - 1),
                                perf_mode=PM.DoubleRow)
                            n += 1
                    if half == 0:
                        nc.vector.tensor_scalar(
                            h1p[half][j // 2][:, j % 2, :], ps,
                            b1_sb[:, j:j + 1], 0.0, ALU.add, ALU.max)
                    else:
                        nc.scalar.activation(
                            h1p[half][j // 2][:, j % 2, :], ps, AF.Relu,
                            bias=b1_sb[:, j:j + 1])

                def emit_w2(half, m, blocks=(0, 1)):
                    # transposed output: h1p is the stationary operand, so
                    # ffT arrives as [token, feature] -- no transpose needed.
                    for bb in blocks:
                        blk = 2 * half + bb
                        ff = fring.tile([128, 512], F32, tag="fring",
                                        name=f"ff_{half}_{m}_{bb}")
                        n = 0
                        nmm = 16 * NW2
                        for sp in range(NW2):
                            w2v = w2q[(half, m, sp)]
                            for u in range(16):
                                nc.tensor.matmul(
                                    ff[:, 0:128],
                                    h1p[half][u][:, :,
                                                 bb * 128:(bb + 1) * 128],
                                    w2v[:, u, :, :],
                                    start=(n == 0), stop=(n == nmm - 1),
                                    perf_mode=PM.DoubleRow)
                                n += 1
                        yb = y1[blk]
                        # y2 = z + ff/2048 where z = y1*rstd1 + b2 (or
                        # x1 + b2 in the affine path), precomputed per block
                        nc.vector.scalar_tensor_tensor(
                            yb[:, m * 128:(m + 1) * 128],
                            ff[:, 0:128], 1.0 / 2048.0,
                            z2[blk][:, m * 128:(m + 1) * 128],
                            ALU.mult, ALU.add)

                # ---- schedule ----
                def extra_h0(v):
                    emit_qproj2(2 * v)
                    emit_qproj2(2 * v + 1)

                def emit_qproj1_unit(G, n, eo):
                    ps = fring.tile([128, 512], F32, tag="fring",
                                    name=f"q1_{G}_{n}_{eo}")
                    for k in range(4):
                        nc.tensor.matmul(
                            ps[:],
                            wq1_sb[G][:, k, :, eo * 128:(eo + 1) * 128],
                            xtp_sb[k][:, :, n * 512:(n + 1) * 512],
                            start=(k == 0), stop=(k == 3),
                            perf_mode=PM.DoubleRow)
                    nc.vector.tensor_scalar_add(
                        qp[G][:, eo, n * 512:(n + 1) * 512], ps[:],
                        bqp_sb[:, 2 * G + eo:2 * G + eo + 1])

                def extra_g(G):
                    def f(v):
                        for n in (2 * (v % 2), 2 * (v % 2) + 1):
                            for eo in range(2):
                                if v < 2 or n >= 2:
                                    pass
                        # two units per quad: (n, eo) pairs in quad order
                        units = [(0, 0), (0, 1), (1, 0), (1, 1),
                                 (2, 0), (2, 1), (3, 0), (3, 1)]
                        for n, eo in units[2 * v:2 * v + 2]:
                            emit_qproj1_unit(G, n, eo)
                    return f

                with (
                    tc.tile_pool(name="ld", bufs=1) as ld,
                ):
                    # PE warm-up: the cost model runs the PE at 0.65/1.2GHz
                    # until it has been continuously busy for 3us. Burn the
                    # initial DMA wait on dependency-free transposes so the
                    # real qproj/scores start at full clock.
                    for i in range(20):
                        wt = fring.tile([128, 512], F32, tag="fring",
                                        name=f"warm{i}")
                        nc.tensor.transpose(wt[:, 0:128], idnf[:], idnf[:])
                    # ---- input loads (latency-ordered) ----
                    t = ld.tile([128, 4, 2, 256], FP8, name="wq1_0")
                    nc.sync.dma_start(t[:], wq1_d[0, :, :, :, :])
                    wq1_sb.append(t)
                    for k in range(4):
                        t = ld.tile([128, 2, S], FP8, name=f"xtp{k}")
                        (nc.sync if k % 2 == 0 else nc.scalar).dma_start(
                            t[:, :, 0:512], xtp_d[:, k, :, 0:512])
                        xtp_sb.append(t)
                    for k in range(4):
                        (nc.sync if k % 2 == 0 else nc.scalar).dma_start(
                            xtp_sb[k][:, :, 512:S], xtp_d[:, k, :, 512:S])
                    t = ld.tile([128, 4, 2, D], FP8, name="wqm")
                    nc.scalar.dma_start(t[:], wqm_d[:])
                    wqm_sb.append(t)
                    for G in range(1, 4):
                        t = ld.tile([128, 4, 2, 256], FP8, name=f"wq1_{G}")
                        nc.sync.dma_start(t[:], wq1_d[G, :, :, :, :])
                        wq1_sb.append(t)
                    # residuals (xq has bq and the pi permutation folded in)
                    for blk in range(4):
                        nc.sync.dma_start(y1[blk][:],
                                          xq_d[blk * 128:(blk + 1) * 128, :])
                    nc.sync.dma_start(b1_sb[:], b1_d[:])
                    nc.sync.dma_start(b2_sb[:], b2_d[:])

                    # half-0 attention heads 0..3, qproj threaded in
                    emit_qproj1(0)
                    if STAGE >= 2:
                        emit_attn(0, 0, extra=extra_h0)
                        for h in (1, 2, 3):
                            emit_attn(0, h, extra=extra_g(h))
                    else:
                        extra_h0(0)
                        extra_h0(1)
                        extra_h0(2)
                        extra_h0(3)
                        for G in (1, 2, 3):
                            emit_qproj1(G)
                        for sp in range(NW1):
                            for sl in range(8):
                                nc.scalar.dma_start(w1_sb[sp][sl][:],
                                                    w1_d[sp, sl, :, :])

                if True:
                    if STAGE >= 2:
                        for h in range(4, H):
                            if 4 <= h < 4 + 8 * NW1:
                                sp, sl = divmod(h - 4, 8)
                                nc.scalar.dma_start(w1_sb[sp][sl][:],
                                                    w1_d[sp, sl, :, :])
                            emit_attn(0, h)
                    SUB = "all"
                    flush_epi()
                    if STAGE >= 3:
                        emit_ln(0, ln2=False)
                    # half-1 attention with half-0 FFN interleaved (the
                    # FFN chunk is emitted mid-head, after quad 2's exp, so
                    # its dep-parked MMs drain while ACT chews quads 2-3)
                    def ffn0_chunk(h, part):
                        if STAGE < 3:
                            return
                        if h == 0:
                            # spread x1tp(0) + z2(0) into head 0's quads so
                            # its dep-parked transposes never block the SEQ
                            if SUB in ("x1tp", "w1", "w2", "all"):
                                emit_x1tp_chunk(0, 2 * part)
                                emit_x1tp_chunk(0, 2 * part + 1)
                                if part == 1:
                                    emit_z2(0)
                            return
                        if h < 9:
                            if part == 0:
                                if h == 7 and SUB in ("w2", "all"):
                                    w2_fetch(0, 0)
                                if h == 8 and SUB in ("w2", "all"):
                                    w2_fetch(0, 1)
                            if SUB in ("w1", "w2", "all"):
                                for j in range(4 * (h - 1) + 2 * part,
                                               4 * (h - 1) + 2 * part + 2):
                                    emit_w1(0, j)
                        else:
                            if SUB in ("w2", "all"):
                                if part == 0 and h < 15:
                                    w2_fetch(0, h - 7)
                                emit_w2(0, h - 9, blocks=(part,))

                    for h in range(H):
                        if STAGE >= 2:
                            done = [0]

                            def mid(v, h=h, done=done):
                                if v in (2, 3) and done[0] == v - 2:
                                    ffn0_chunk(h, v - 2)
                                    done[0] = v - 1
                            emit_attn(1, h, extra=mid)
                            while done[0] < 2:
                                ffn0_chunk(h, done[0])
                                done[0] += 1
                        else:
                            ffn0_chunk(h, 0)
                            ffn0_chunk(h, 1)
                    if STAGE >= 3 and SUB in ("w2", "all"):
                        emit_w2(0, 7)
                    if STAGE >= 4:
                        flush_epi()
                        emit_ln(0, ln2=True)
                        emit_ln(1, ln2=False, tail=True)
                        for c in range(4):
                            emit_x1tp_chunk(1, c)
                        emit_z2(1)
                        for j in range(32):
                            if j % 2 == 0 and j < 8:
                                w2_fetch(1, j // 2)
                            emit_w1(1, j)
                        for m in range(8):
                            if m < 4:
                                w2_fetch(1, m + 4)
                            emit_w2(1, m)
                        emit_ln(1, ln2=True, tail=True)
                    else:
                        # sink: store y1 so the module has outputs
                        for blk in range(4):
                            nc.sync.dma_start(
                                out_d[blk * 128:(blk + 1) * 128, :],
                                y1[blk][:])

    nc.compile()
    _BUILD_CACHE[apply_affine] = nc
    return nc


# ---- host-side packing ----

_PI = np.empty(D, np.int64)
for _h in range(H):
    for _eo in range(2):
        for _j in range(32):
            _PI[_h * 64 + _eo * 32 + _j] = _h * 64 + 2 * _j + _eo


def _f8(a):
    f8 = dt.np(FP8)
    return np.ascontiguousarray(np.asarray(a, np.float32)).astype(f8)


def kernel(x, Wq, bq, ln1_g, ln1_b, W1, b1, W2, b2, ln2_g, ln2_b):
    x = np.asarray(x, np.float32)
    trivial = (np.all(ln1_g == 1) and np.all(ln1_b == 0)
               and np.all(ln2_g == 1) and np.all(ln2_b == 0))
    nc = _build(apply_affine=not trivial)

    Wqf = np.asarray(Wq, np.float32).transpose(1, 0, 2).reshape(D, D)
    bqf = np.asarray(bq, np.float32).reshape(D)
    W1 = np.asarray(W1, np.float32)
    W2 = np.asarray(W2, np.float32)
    b1 = np.asarray(b1, np.float32)
    b2 = np.asarray(b2, np.float32)

    # wq1[G, p, k, i, eo*128 + 32a + j] = Wqf[256k+128i+p, (4G+a)*64 + 2j+eo]
    wq_r = Wqf.reshape(4, 2, 128, H, 32, 2)      # [k, i, p, h, j, eo]
    wq1 = np.transpose(wq_r.reshape(4, 2, 128, 4, 4, 32, 2),
                       (3, 2, 0, 1, 6, 4, 5))    # [G, p, k, i, eo, a, j]
    wq1 = _f8(32.0 * wq1.reshape(4, 128, 4, 2, 256))
    # wqm[p, k, i, f] = Wqf[256k+128i+p, PI[f]]
    wqm = _f8(32.0 * np.transpose(
        Wqf[:, _PI].reshape(4, 2, 128, D), (2, 0, 1, 3)))
    # bqp[32a+j, 2G+eo] = bqf[(4G+a)*64 + 2j+eo]
    bq_r = bqf.reshape(4, 4, 32, 2)              # [G, a, j, eo]
    bqp = np.ascontiguousarray(
        32.0 * np.transpose(bq_r, (1, 2, 0, 3)).reshape(128, 8))

    def wsplit(w, n, scale):
        f8 = dt.np(FP8)
        w = w * scale
        hi = w.astype(f8)
        if n == 1:
            return [hi]
        lo = (w - hi.astype(np.float32)).astype(f8)
        return [hi, lo]

    # w1p[sp, sl, p, (jj c i m)] = W1[PI[256c+128i+p], 128(4sl+jj)+m]
    w1perm = W1[_PI, :]                          # [d(pi), HID]
    w1r = w1perm.reshape(4, 2, 128, 32, 128)     # [c, i, p, j, m]
    w1r = np.transpose(w1r, (2, 3, 0, 1, 4))     # [p, j, c, i, m]
    w1r = np.transpose(w1r.reshape(128, 8, 4, 4, 2, 128),
                       (1, 0, 2, 3, 4, 5))       # [sl, p, jj, c, i, m]
    w1ps = [np.ascontiguousarray(w.reshape(8, 128, 4096))
            for w in wsplit(w1r, NW1, 32.0)]
    # w2p[sp, m, p, (u i mm)] = W2[128(2u+i)+p, PI[128m+mm]]
    w2perm = W2[:, _PI]
    w2r = w2perm.reshape(16, 2, 128, 8, 128)     # [u, i, p, m, mm]
    w2r = np.transpose(w2r, (3, 2, 0, 1, 4))     # [m, p, u, i, mm]
    w2ps = [np.ascontiguousarray(w.reshape(8, 128, 4096))
            for w in wsplit(w2r, NW2, 64.0)]

    base = {
        "wq1": wq1,
        "wqm": wqm,
        "bqp": bqp,
        "w1p": np.stack(w1ps),
        "b1r": np.ascontiguousarray(32.0 * b1.reshape(32, 128).T),
        "w2p": np.stack(w2ps),
        "b2bc": np.ascontiguousarray(
            np.broadcast_to(b2[_PI], (128, D))),
    }
    if not trivial:
        for name, v in (("g1b", np.asarray(ln1_g, np.float32)[_PI]),
                        ("be1b", np.asarray(ln1_b, np.float32)[_PI]),
                        ("g2b", np.asarray(ln2_g, np.float32)[_PI]),
                        ("be2b", np.asarray(ln2_b, np.float32)[_PI])):
            base[name] = np.ascontiguousarray(
                np.broadcast_to(v, (128, D)))

    in_maps = []
    for c in range(NCORES):
        b, t = divmod(c, 4)
        xb = np.concatenate([x[b, t * SQ:], x[b, :t * SQ]], axis=0)
        # xtp[p, k, i, t] = xb[t, 256k+128i+p]
        xtp = _f8(np.transpose(xb.T.reshape(4, 2, 128, S), (2, 0, 1, 3)))
        xq = np.ascontiguousarray(xb[:SQ][:, _PI] + bqf[_PI][None, :])
        in_maps.append({**base, "xtp": xtp, "xq": xq})

    trace = bool(int(os.environ.get("KERNEL_TRACE", "0")))
    kw = {}
    if trace:
        kw = dict(trace=True,
                  tmpdir=os.environ.get("KERNEL_TRACE_DIR") or None)
    res = run_bass_kernel_spmd(nc, in_maps, core_ids=list(range(NCORES)),
                               **kw)
    if trace:
        print(f"HW exec time: {res.exec_time_ns} ns  "
              f"(mean {res.mean_exec_time_ns}, "
              f"max core {res.max_exec_time_core_id})")
    out = np.empty((B, S, D), np.float32)
    inv = np.empty((SQ, D), np.float32)
    for c in range(NCORES):
        b, t = divmod(c, 4)
        inv[:, _PI] = res.results[c]["out_q"]
        out[b, t * SQ:(t + 1) * SQ] = inv
    return out
